# revision 14
# baseline (speedup 1.0000x reference)
"""LinearSelfAttention kernel for TRN2 (8 NeuronCores, batch-parallel).

Key identity: with Hn = H[:, :n] (mask drops column n from the s-sum),
    attn = P H mask(H^T Q H) = C H,   C = P G Q,   G = Hn Hn^T  (257x257)
so  out = H + C H / n = Et^T H,  Et = I + Q^T G P^T / n.
O(n d^2) for G and Et^T H plus O(d^3) for the tiny chain, vs O(3 n d^2)
for the naive re-association.

PE strategy:
 - G via fp8e4m3 DoubleRow (K=256/pass, 0.5 cyc/row), host-transposed Hn.
 - Final Et^T H: everything scaled by S=512 so the +H identity stays an
   exact power-of-2 in bf16 while the attn part runs fp8 DoubleRow
   (Ct*512 is in fp8 normal range); eviction rescales by exact 1/512.
 - The three K=1 matmuls from the e=256 row are packed into disjoint
   32-row PE strips (tile_position) so they pipeline ~concurrently.
 - Output row d=256 (M=1 chunks) goes PSUM->HBM as fp32; host divides.

Sharding: data-parallel over batch, 2 samples per core.
"""

import sys

sys.path.insert(0, "/opt/trn_rl_repo")

import numpy as np
import ml_dtypes

B, D1, N1 = 16, 257, 2049  # batch, d+1, n+1
N = N1 - 1  # 2048
NCORES = 8
BPC = B // NCORES  # samples per core
S = 512.0  # power-of-2 scale keeping fp8 Ct in normal range

# partition chunking of the 257-sized dims: (offset, size)
CH = [(0, 128), (128, 128), (256, 1)]
NT8 = N // 256  # 8 double-row s-tiles of the transposed Hn
DPAD = 272  # fp8 DR LDWEIGHTS: step between the 2 K-subtiles must be %16==0
TPAD = 2064  # DR moving-operand subtile step, %16==0
TCH = [(i * 512, min(512, N1 - i * 512)) for i in range((N1 + 511) // 512)]
NWARM = 8

_cached = {}


def _build():
    import concourse.bass as bass
    import concourse.tile as tile
    from concourse import bacc, mybir
    from contextlib import ExitStack

    f32 = mybir.dt.float32
    bf16 = mybir.dt.bfloat16
    f8 = mybir.dt.float8e4
    DR = mybir.MatmulPerfMode.DoubleRow

    nc = bacc.Bacc("TRN2", target_bir_lowering=False, debug=False, num_devices=NCORES)

    Hb_d = nc.declare_dram_parameter("Hb", [BPC, D1, N1], bf16, isOutput=False)
    Ht_d = nc.declare_dram_parameter("Ht", [BPC, NT8, 128, 2, DPAD], f8, isOutput=False)
    H8_d = nc.declare_dram_parameter("H8", [BPC, 128, 2, TPAD], f8, isOutput=False)
    QPI_d = nc.declare_dram_parameter("QPI", [D1, 3 * D1], bf16, isOutput=False)
    Y_d = nc.declare_dram_parameter("Y", [BPC, D1, N1], bf16, isOutput=True)

    with tile.TileContext(nc) as tc:
        with ExitStack() as ctx:
            const = ctx.enter_context(tc.tile_pool(name="const", bufs=1))
            htp = ctx.enter_context(tc.tile_pool(name="htp", bufs=2))
            hbp = ctx.enter_context(tc.tile_pool(name="hbp", bufs=2))
            sq = ctx.enter_context(tc.tile_pool(name="sq", bufs=2))
            yp = ctx.enter_context(tc.tile_pool(name="yp", bufs=2))

            # ---- input DMAs round-robined over three queues; per-queue FIFO
            # keeps sample 0 ahead of sample 1
            qs = [nc.sync, nc.scalar, nc.gpsimd]
            qi = 0

            def load(dst, src):
                nonlocal qi
                qs[qi % 3].dma_start(dst, src)
                qi += 1

            ht = [[None] * NT8 for _ in range(BPC)]
            hb = [[None] * 2 for _ in range(BPC)]
            h8 = [None] * BPC
            hr = [None] * BPC
            qpi = []
            for b in range(BPC):
                for st in range(NT8):
                    t = htp.tile([128, 2, DPAD], f8, tag=f"ht{st}", name=f"ht{b}_{st}")
                    load(t[:, :, :], Ht_d[b, st])
                    ht[b][st] = t
                if b == 0:  # consts needed only from the V stage onward
                    for c, (off, sz) in enumerate(CH):
                        t = const.tile(
                            [128, 3 * D1], bf16, tag=f"qpi{c}", name=f"qpi{c}"
                        )
                        load(t[:sz, :], QPI_d[off : off + sz, :])
                        qpi.append(t)
                for c in range(2):
                    t = hbp.tile([128, N1], bf16, tag=f"hb{c}", name=f"hb{b}_{c}")
                    load(t[:, :], Hb_d[b, c * 128 : (c + 1) * 128, :])
                    hb[b][c] = t
                t = hbp.tile([128, 2, TPAD], f8, tag="h8", name=f"h8_{b}")
                load(t[:, :, :], H8_d[b])
                h8[b] = t
                # H row e=256 replicated at partitions 0/32 for the
                # packed K=1 edge matmuls
                t = hbp.tile([128, N1], bf16, tag="hr", name=f"hr{b}")
                for k in range(2):
                    load(t[32 * k : 32 * k + 1, :], Hb_d[b, 256:257, :])
                hr[b] = t

            # ---- PE warmup: ride the clock ramp until the first tile lands
            wsb = const.tile([128, 128], bf16, tag="wsb", name="wsb")
            nc.vector.memset(wsb[:, :], 0.0)
            with tc.tile_pool(name="wp", bufs=1, space="PSUM") as wp:
                wps = wp.tile([128, 512], f32, tag="wps", name="warm_ps")
                for i in range(NWARM):
                    nc.tensor.matmul(
                        wps[:, 0:128],
                        wsb[:, :],
                        wsb[:, :],
                        start=(i == 0),
                        stop=(i == NWARM - 1),
                    )

            with (
                tc.tile_pool(name="ppg", bufs=1, space="PSUM") as ppg,
                tc.tile_pool(name="pp4", bufs=2, space="PSUM") as pp4,
            ):
                for b in range(BPC):
                    # ---- G = Hn Hn^T (fp8 DoubleRow, K=256 per pass)
                    g_ps = [
                        ppg.tile([128, D1], f32, tag=f"g{ac}", name=f"g_ps{b}_{ac}")
                        for ac in range(3)
                    ]
                    for st in range(NT8):
                        for ac, (aoff, asz) in enumerate(CH):
                            nc.tensor.matmul(
                                g_ps[ac][:asz, :],
                                ht[b][st][:, :, aoff : aoff + asz],
                                ht[b][st][:, :, :D1],
                                start=(st == 0),
                                stop=(st == NT8 - 1),
                                perf_mode=DR,
                            )
                    gsb = []
                    for ac, (aoff, asz) in enumerate(CH):
                        t = sq.tile([128, D1], bf16, tag=f"g{ac}", name=f"gs{b}_{ac}")
                        eng = nc.scalar.copy if ac % 2 == 0 else nc.vector.tensor_copy
                        eng(t[:asz, :], g_ps[ac][:asz, :])
                        gsb.append(t)

                    # ---- V = G P^T  (G symmetric: lhsT slices G directly)
                    v_ps = [
                        ppg.tile([128, D1], f32, tag=f"g{am}", name=f"v_ps{b}_{am}")
                        for am in range(3)
                    ]
                    for am, (amoff, amsz) in enumerate(CH):
                        for kb, (kboff, kbsz) in enumerate(CH):
                            nc.tensor.matmul(
                                v_ps[am][:amsz, :],
                                gsb[kb][:kbsz, amoff : amoff + amsz],
                                qpi[kb][:kbsz, D1 : 2 * D1],
                                start=(kb == 0),
                                stop=(kb == 2),
                            )
                    vsb = []
                    for am, (amoff, amsz) in enumerate(CH):
                        t = sq.tile([128, D1], bf16, tag=f"v{am}", name=f"vs{b}_{am}")
                        eng = nc.scalar.copy if am % 2 == 1 else nc.vector.tensor_copy
                        eng(t[:amsz, :], v_ps[am][:amsz, :])
                        vsb.append(t)

                    # ---- Ct = (S Q/n)^T V  (= S C^T/n), fp32 in PSUM
                    c_ps = [
                        ppg.tile([128, D1], f32, tag=f"g{em}", name=f"c_ps{b}_{em}")
                        for em in range(3)
                    ]
                    for em, (emoff, emsz) in enumerate(CH):
                        for ka, (kaoff, kasz) in enumerate(CH):
                            nc.tensor.matmul(
                                c_ps[em][:emsz, :],
                                qpi[ka][:kasz, emoff : emoff + emsz],
                                vsb[ka][:kasz, :],
                                start=(ka == 0),
                                stop=(ka == 2),
                            )
                    # DR weights for the attn part: e rows 0..255 packed [p,2,d]
                    ct8 = sq.tile([128, 2, DPAD], f8, tag="ct8", name=f"ct8_{b}")
                    nc.scalar.copy(ct8[:, 0, :D1], c_ps[0][:, :])
                    nc.vector.tensor_copy(ct8[:, 1, :D1], c_ps[1][:, :])
                    # e=256 row at partitions 0/32 for the packed edges
                    er = sq.tile([128, D1], bf16, tag="er", name=f"er{b}")
                    nc.scalar.copy(er[0:1, :], c_ps[2][0:1, :])
                    nc.sync.dma_start(er[32:33, :], er[0:1, :])

                    # ---- Y = Et^T H, scaled by S in PSUM
                    for ti, (toff, tsz) in enumerate(TCH):
                        p0 = pp4.tile([128, 512], f32, tag="pa", name=f"pa{b}_{ti}")
                        p1 = pp4.tile([128, 512], f32, tag="pb", name=f"pb{b}_{ti}")
                        for dc in range(2):
                            p = (p0, p1)[dc]
                            nc.tensor.matmul(
                                p[:128, :tsz],
                                ct8[:, :, dc * 128 : (dc + 1) * 128],
                                h8[b][:, :, toff : toff + tsz],
                                start=True,
                                stop=False,
                                perf_mode=DR,
                            )
                            nc.tensor.matmul(
                                p[:128, :tsz],
                                qpi[dc][
                                    :128, 2 * D1 + dc * 128 : 2 * D1 + (dc + 1) * 128
                                ],
                                hb[b][dc][:, toff : toff + tsz],
                                start=False,
                                stop=False,
                            )
                        # K=1 edges, packed into PE row strips 0/32/64
                        nc.tensor.matmul(
                            p0[:128, :tsz],
                            er[0:1, 0:128],
                            hr[b][0:1, toff : toff + tsz],
                            start=False,
                            stop=True,
                        )
                        nc.tensor.matmul(
                            p1[:128, :tsz],
                            er[32:33, 128:256],
                            hr[b][32:33, toff : toff + tsz],
                            start=False,
                            stop=True,
                        )
                        # evict with the exact 1/S rescale; row 256 raw to HBM
                        y0 = yp.tile([128, N1], bf16, tag="y0", name=f"y0_{b}")
                        y1 = yp.tile([128, N1], bf16, tag="y1", name=f"y1_{b}")
                        nc.scalar.mul(y0[:, toff : toff + tsz], p0[:128, :tsz], 1.0 / S)
                        nc.vector.tensor_scalar_mul(
                            y1[:, toff : toff + tsz], p1[:128, :tsz], 1.0 / S
                        )
                        nc.sync.dma_start(
                            Y_d[b, 0:128, toff : toff + tsz],
                            y0[:, toff : toff + tsz],
                        )
                        nc.gpsimd.dma_start(
                            Y_d[b, 128:256, toff : toff + tsz],
                            y1[:, toff : toff + tsz],
                        )

    nc.compile()
    return nc


def _prep_in_maps(H, P, Q):
    bf = ml_dtypes.bfloat16
    f8 = ml_dtypes.float8_e4m3
    H = np.ascontiguousarray(H, dtype=np.float32)
    Hb = H.astype(bf)
    # G operand: [st, p, i, d] with s = st*256 + i*128 + p
    Ht = np.swapaxes(H[:, :, :N], 1, 2).reshape(B, NT8, 2, 128, D1)
    Ht8 = np.zeros((B, NT8, 128, 2, DPAD), dtype=f8)
    Ht8[..., :D1] = np.swapaxes(Ht, 2, 3).astype(f8)
    # attn stream operand: [p, i, t] with e = i*128 + p
    H8 = np.zeros((B, 128, 2, TPAD), dtype=f8)
    H8[..., :N1] = np.swapaxes(H[:, :256, :].reshape(B, 2, 128, N1), 1, 2).astype(f8)
    QPI = np.concatenate(
        [Q * (S / N), P.T, S * np.eye(D1, dtype=np.float32)], axis=1
    ).astype(bf)
    QPI = np.ascontiguousarray(QPI)
    return [
        {
            "Hb": Hb[c * BPC : (c + 1) * BPC],
            "Ht": Ht8[c * BPC : (c + 1) * BPC],
            "H8": H8[c * BPC : (c + 1) * BPC],
            "QPI": QPI,
        }
        for c in range(NCORES)
    ]


def kernel(H, P, Q):
    from concourse.bass_utils import run_bass_kernel_spmd

    if "nc" not in _cached:
        _cached["nc"] = _build()
    nc = _cached["nc"]

    in_maps = _prep_in_maps(H, P, Q)
    res = run_bass_kernel_spmd(nc, in_maps, list(range(NCORES)))
    out = np.concatenate(
        [res.results[c]["Y"].astype(np.float32) for c in range(NCORES)], axis=0
    )
    # output row d=256 exactly, on host (fp32): avoids the M=1 PE chunks
    H = np.ascontiguousarray(H, dtype=np.float32)
    Hn = H[:, :, :N]
    u = np.einsum("bds,d->bs", Hn, np.ascontiguousarray(P[256, :], np.float32))
    v = np.einsum("bds,bs->bd", Hn, u)  # = G @ P[256,:] per sample
    c256 = v @ Q  # = C[256, :] per sample
    out[:, 256, :] = H[:, 256, :] + np.einsum("bd,bdt->bt", c256, H) / N
    return out


# revision 18
# speedup vs baseline: 1.1124x; 1.1124x over previous
"""LinearSelfAttention kernel for TRN2 (8 NeuronCores, batch-parallel).

Key identity: with Hn = H[:, :n] (mask drops column n from the s-sum),
    attn = P H mask(H^T Q H) = C H,   C = P G Q,   G = Hn Hn^T  (257x257)
so  out = H + C H / n = Et^T H,  Et = I + Q^T G P^T / n.
O(n d^2) for G and Et^T H plus O(d^3) for the tiny chain, vs O(3 n d^2)
for the naive re-association.

PE strategy:
 - G via fp8e4m3 DoubleRow (K=256/pass, 0.5 cyc/row), host-transposed Hn.
 - Final Et^T H in bf16 with the +H identity folded into Et (FWL keeps
   LDWEIGHTS hidden; fp8 DoubleRow loses FWL and is a net loss here).
 - The K=1 matmuls from the e=256 row are packed into disjoint 32-row
   PE strips (tile_position) so they pipeline back-to-back.
 - Output row d=256 (the M=1 edge) is computed exactly on the host.
 - DMAs batched aggressively: each DMA_DIRECT2D costs ~0.6us of engine
   issue time, so fewer/bigger transfers win.

Sharding: data-parallel over batch, 2 samples per core.
"""

import sys

sys.path.insert(0, "/opt/trn_rl_repo")

import numpy as np
import ml_dtypes

B, D1, N1 = 16, 257, 2049  # batch, d+1, n+1
N = N1 - 1  # 2048
NCORES = 8
BPC = B // NCORES  # samples per core

# partition chunking of the 257-sized dims: (offset, size)
CH = [(0, 128), (128, 128), (256, 1)]
NT8 = N // 256  # 8 double-row s-tiles of the transposed Hn
DPAD = 272  # fp8 DR LDWEIGHTS: step between the 2 K-subtiles must be %16==0
TCH = [(i * 512, min(512, N1 - i * 512)) for i in range((N1 + 511) // 512)]
NWARM = 14

_cached = {}


def _build():
    import concourse.bass as bass
    import concourse.tile as tile
    from concourse import bacc, mybir
    from contextlib import ExitStack

    f32 = mybir.dt.float32
    bf16 = mybir.dt.bfloat16
    f8 = mybir.dt.float8e4
    DR = mybir.MatmulPerfMode.DoubleRow

    nc = bacc.Bacc("TRN2", target_bir_lowering=False, debug=False, num_devices=NCORES)

    Hb_d = nc.declare_dram_parameter("Hb", [BPC, D1, N1], bf16, isOutput=False)
    Ht_d = nc.declare_dram_parameter("Ht", [BPC, NT8, 128, 2, DPAD], f8, isOutput=False)
    QPI_d = nc.declare_dram_parameter("QPI", [D1, 3 * D1], bf16, isOutput=False)
    Y_d = nc.declare_dram_parameter("Y", [BPC, 256, N1], bf16, isOutput=True)

    with tile.TileContext(nc) as tc:
        with ExitStack() as ctx:
            const = ctx.enter_context(tc.tile_pool(name="const", bufs=1))
            htp = ctx.enter_context(tc.tile_pool(name="htp", bufs=2))
            hbp = ctx.enter_context(tc.tile_pool(name="hbp", bufs=2))
            sq = ctx.enter_context(tc.tile_pool(name="sq", bufs=2))
            yp = ctx.enter_context(tc.tile_pool(name="yp", bufs=2))

            # ---- input DMAs, batched; per-queue FIFO keeps sample 0 first
            ht = [None] * BPC
            hb = [None] * BPC
            hr = [None] * BPC
            qpi = []
            for b in range(BPC):
                # all 8 DR s-tiles of Hn^T in one transfer
                t = htp.tile([128, NT8, 2, DPAD], f8, tag="ht", name=f"ht{b}")
                nc.sync.dma_start(t[:, :, :, :], Ht_d[b])
                ht[b] = t
                if b == 0:  # consts needed only from the V stage onward
                    for c, (off, sz) in enumerate(CH):
                        t = const.tile(
                            [128, 3 * D1], bf16, tag=f"qpi{c}", name=f"qpi{c}"
                        )
                        nc.gpsimd.dma_start(t[:sz, :], QPI_d[off : off + sz, :])
                        qpi.append(t)
                # H rows 0..255: [p, c, t] = H[c*128+p, t]
                t = hbp.tile([128, 2, N1], bf16, tag="hb", name=f"hb{b}")
                for c in range(2):
                    nc.scalar.dma_start(t[:, c, :], Hb_d[b, c * 128 : (c + 1) * 128, :])
                hb[b] = t
                # H row e=256 replicated at partitions 0/32 for the edges
                t = hbp.tile([128, N1], bf16, tag="hr", name=f"hr{b}")
                nc.gpsimd.dma_start(t[0:1, :], Hb_d[b, 256:257, :])
                nc.gpsimd.dma_start(t[32:33, :], Hb_d[b, 256:257, :])
                hr[b] = t

            # ---- PE warmup: ride the clock ramp until the first tile lands
            wsb = const.tile([128, 128], bf16, tag="wsb", name="wsb")
            nc.vector.memset(wsb[:, :], 0.0)
            with tc.tile_pool(name="wp", bufs=1, space="PSUM") as wp:
                wps = wp.tile([128, 512], f32, tag="wps", name="warm_ps")
                for i in range(NWARM):
                    nc.tensor.matmul(
                        wps[:, 0:128],
                        wsb[:, :],
                        wsb[:, :],
                        start=(i == 0),
                        stop=(i == NWARM - 1),
                    )

            with (
                tc.tile_pool(name="ppg", bufs=1, space="PSUM") as ppg,
                tc.tile_pool(name="pp4", bufs=2, space="PSUM") as pp4,
            ):
                for b in range(BPC):
                    # ---- G = Hn Hn^T (fp8 DoubleRow, K=256 per pass)
                    g_ps = [
                        ppg.tile([128, D1], f32, tag=f"g{ac}", name=f"g_ps{b}_{ac}")
                        for ac in range(3)
                    ]
                    for st in range(NT8):
                        for ac, (aoff, asz) in enumerate(CH):
                            nc.tensor.matmul(
                                g_ps[ac][:asz, :],
                                ht[b][:, st, :, aoff : aoff + asz],
                                ht[b][:, st, :, :D1],
                                start=(st == 0),
                                stop=(st == NT8 - 1),
                                perf_mode=DR,
                            )
                    gsb = []
                    for ac, (aoff, asz) in enumerate(CH):
                        t = sq.tile([128, D1], bf16, tag=f"g{ac}", name=f"gs{b}_{ac}")
                        eng = nc.scalar.copy if ac % 2 == 0 else nc.vector.tensor_copy
                        eng(t[:asz, :], g_ps[ac][:asz, :])
                        gsb.append(t)

                    # ---- V = G P^T  (G symmetric: lhsT slices G directly)
                    v_ps = [
                        ppg.tile([128, D1], f32, tag=f"g{am}", name=f"v_ps{b}_{am}")
                        for am in range(3)
                    ]
                    for am, (amoff, amsz) in enumerate(CH):
                        for kb, (kboff, kbsz) in enumerate(CH):
                            nc.tensor.matmul(
                                v_ps[am][:amsz, :],
                                gsb[kb][:kbsz, amoff : amoff + amsz],
                                qpi[kb][:kbsz, D1 : 2 * D1],
                                start=(kb == 0),
                                stop=(kb == 2),
                            )
                    vsb = []
                    for am, (amoff, amsz) in enumerate(CH):
                        t = sq.tile([128, D1], bf16, tag=f"v{am}", name=f"vs{b}_{am}")
                        eng = nc.scalar.copy if am % 2 == 1 else nc.vector.tensor_copy
                        eng(t[:amsz, :], v_ps[am][:amsz, :])
                        vsb.append(t)

                    # ---- Ct = (Q/n)^T V  (= C^T/n), then Et = I + Ct
                    c_ps = [
                        ppg.tile([128, D1], f32, tag=f"g{em}", name=f"c_ps{b}_{em}")
                        for em in range(3)
                    ]
                    for em, (emoff, emsz) in enumerate(CH):
                        for ka, (kaoff, kasz) in enumerate(CH):
                            nc.tensor.matmul(
                                c_ps[em][:emsz, :],
                                qpi[ka][:kasz, emoff : emoff + emsz],
                                vsb[ka][:kasz, :],
                                start=(ka == 0),
                                stop=(ka == 2),
                            )
                    et = []
                    for em in range(2):
                        t = sq.tile([128, D1], bf16, tag=f"e{em}", name=f"et{b}_{em}")
                        nc.vector.tensor_add(
                            t[:, :],
                            c_ps[em][:, :],
                            qpi[em][:, 2 * D1 : 3 * D1],
                        )
                        et.append(t)
                    # e=256 row (= Ct[256,:], identity lives at d=256 only,
                    # which the host computes) at partitions 0/32
                    er = sq.tile([128, D1], bf16, tag="er", name=f"er{b}")
                    nc.scalar.copy(er[0:1, :], c_ps[2][0:1, :])
                    nc.scalar.dma_start(er[32:33, :], er[0:1, :])

                    # ---- Y[d<256] = Et^T H
                    y = [
                        yp.tile([128, N1], bf16, tag=f"y{dc}", name=f"y{b}_{dc}")
                        for dc in range(2)
                    ]
                    for ti, (toff, tsz) in enumerate(TCH):
                        p0 = pp4.tile([128, 512], f32, tag="pa", name=f"pa{b}_{ti}")
                        p1 = pp4.tile([128, 512], f32, tag="pb", name=f"pb{b}_{ti}")
                        for dc, p in ((0, p0), (1, p1)):
                            dsl = slice(dc * 128, (dc + 1) * 128)
                            for ec in range(2):
                                nc.tensor.matmul(
                                    p[:128, :tsz],
                                    et[ec][:, dsl],
                                    hb[b][:, ec, toff : toff + tsz],
                                    start=(ec == 0),
                                    stop=False,
                                )
                        # K=1 edges, packed into PE row strips 0/32
                        nc.tensor.matmul(
                            p0[:128, :tsz],
                            er[0:1, 0:128],
                            hr[b][0:1, toff : toff + tsz],
                            start=False,
                            stop=True,
                        )
                        nc.tensor.matmul(
                            p1[:128, :tsz],
                            er[32:33, 128:256],
                            hr[b][32:33, toff : toff + tsz],
                            start=False,
                            stop=True,
                        )
                        nc.scalar.copy(y[0][:, toff : toff + tsz], p0[:128, :tsz])
                        nc.vector.tensor_copy(y[1][:, toff : toff + tsz], p1[:128, :tsz])
                        # store each half-row once its chunks are evicted
                        if ti == 2:
                            nc.sync.dma_start(Y_d[b, 0:128, 0:1024], y[0][:, 0:1024])
                            nc.gpsimd.dma_start(
                                Y_d[b, 128:256, 0:1024], y[1][:, 0:1024]
                            )
                    nc.sync.dma_start(Y_d[b, 0:128, 1024:N1], y[0][:, 1024:N1])
                    nc.gpsimd.dma_start(Y_d[b, 128:256, 1024:N1], y[1][:, 1024:N1])

    nc.compile()
    return nc


def _prep_in_maps(H, P, Q):
    bf = ml_dtypes.bfloat16
    f8 = ml_dtypes.float8_e4m3
    H = np.ascontiguousarray(H, dtype=np.float32)
    Hb = H.astype(bf)
    # G operand: [st, p, i, d] with s = st*256 + i*128 + p
    Ht = np.swapaxes(H[:, :, :N], 1, 2).reshape(B, NT8, 2, 128, D1)
    Ht8 = np.zeros((B, NT8, 128, 2, DPAD), dtype=f8)
    Ht8[..., :D1] = np.swapaxes(Ht, 2, 3).astype(f8)
    QPI = np.concatenate(
        [Q / N, P.T, np.eye(D1, dtype=np.float32)], axis=1
    ).astype(bf)
    QPI = np.ascontiguousarray(QPI)
    return [
        {
            "Hb": Hb[c * BPC : (c + 1) * BPC],
            "Ht": Ht8[c * BPC : (c + 1) * BPC],
            "QPI": QPI,
        }
        for c in range(NCORES)
    ]


def kernel(H, P, Q):
    from concourse.bass_utils import run_bass_kernel_spmd

    if "nc" not in _cached:
        _cached["nc"] = _build()
    nc = _cached["nc"]

    in_maps = _prep_in_maps(H, P, Q)
    res = run_bass_kernel_spmd(nc, in_maps, list(range(NCORES)))
    out = np.empty((B, D1, N1), dtype=np.float32)
    ydev = np.concatenate(
        [res.results[c]["Y"].astype(np.float32) for c in range(NCORES)], axis=0
    )
    out[:, :256, :] = ydev
    # output row d=256 exactly, on host (fp32): avoids the M=1 PE chunks
    H = np.ascontiguousarray(H, dtype=np.float32)
    Hn = H[:, :, :N]
    u = np.einsum("bds,d->bs", Hn, np.ascontiguousarray(P[256, :], np.float32))
    v = np.einsum("bds,bs->bd", Hn, u)  # = G @ P[256,:] per sample
    c256 = v @ Q  # = C[256, :] per sample
    out[:, 256, :] = H[:, 256, :] + np.einsum("bd,bdt->bt", c256, H) / N
    return out
